# revision 7
# baseline (speedup 1.0000x reference)
"""Trainium2 Bass kernel for fused attention block (QKV+gate proj, q/k RMS-norm,
RoPE, causal GQA attention, sigmoid gating, o_proj).

Sharding: 8 cores = 2 batches x 4 head-groups (tensor-parallel over heads,
data-parallel over batch). Each core computes a partial [T, D] output from its
4 q-heads / 1 kv-head; host sums the 4 partials per batch.

Self-contained: hardcodes all shapes; reads nothing from /root/problem.
"""

import os
import numpy as np
import ml_dtypes

import concourse.bass as bass
import concourse.bacc as bacc
import concourse.mybir as mybir
import concourse.tile as tile
from concourse.bass import ts, ds
from concourse.bass_utils import run_bass_kernel_spmd

# ---- problem constants ----
B, T, D = 2, 2048, 2048
NH, NKV, HD = 16, 4, 128
NQ = NH // NKV          # q heads per core
DH = NQ * HD            # 512 (attn feature rows per core)
EPS = 1e-6
SCALE = HD ** -0.5
TB = 512                # moving free-dim block
NTB = T // TB           # 4
NKT = D // 128          # 16 contraction tiles
NTT = T // 128          # 16 t(row)-tiles

F32 = mybir.dt.float32
BF16 = mybir.dt.bfloat16
F32R = mybir.dt.float32r
AF = mybir.ActivationFunctionType
NPBF16 = ml_dtypes.bfloat16

# matmul storage dtype: "bf16" or "f32r"
MM_MODE = os.environ.get("KERNEL_MM_MODE", "bf16")
MMDT = BF16 if MM_MODE == "bf16" else F32
NPMM = NPBF16 if MM_MODE == "bf16" else np.float32


def _mm(nc, out, lhsT, rhs, **kw):
    """matmul that goes through float32r when MM_MODE=f32r."""
    if MM_MODE == "f32r":
        lhsT = lhsT.bitcast(F32R)
        rhs = rhs.bitcast(F32R)
    nc.tensor.matmul(out, lhsT, rhs, **kw)


def _emit(tc, io):
    nc = tc.nc
    with (
        tc.tile_pool(name="consts", bufs=1) as cpool,
        tc.tile_pool(name="persist", bufs=1) as ppool,
        tc.tile_pool(name="xt", bufs=2) as xpool,
        tc.tile_pool(name="workB", bufs=2) as wb,
        tc.tile_pool(name="rows", bufs=3) as rows,
        tc.tile_pool(name="probs", bufs=4) as prp,
        tc.tile_pool(name="workC", bufs=2) as wc,
        tc.tile_pool(name="outp", bufs=2) as op,
        tc.tile_pool(name="ps_sc", bufs=3, space="PSUM") as ps_sc,
        tc.tile_pool(name="ps_acc", bufs=2, space="PSUM") as ps_acc,
        tc.tile_pool(name="ps_misc", bufs=2, space="PSUM") as ps_misc,
        tc.tile_pool(name="ps_den", bufs=1, space="PSUM") as ps_den,
    ):
        # ---------- constants / weights (resident) ----------
        wq_sb = cpool.tile([128, NKT, DH], MMDT, name="wq_sb")
        nc.sync.dma_start(wq_sb[:], io["wq"].rearrange("(ko p) c -> p ko c", p=128))
        wg_sb = cpool.tile([128, NKT, DH], MMDT, name="wg_sb")
        nc.sync.dma_start(wg_sb[:], io["wg"].rearrange("(ko p) c -> p ko c", p=128))
        wk_sb = cpool.tile([128, NKT, HD], MMDT, name="wk_sb")
        nc.sync.dma_start(wk_sb[:], io["wk"].rearrange("(ko p) c -> p ko c", p=128))
        wv_sb = cpool.tile([128, NKT, HD], MMDT, name="wv_sb")
        nc.sync.dma_start(wv_sb[:], io["wv"].rearrange("(ko p) c -> p ko c", p=128))
        wo_sb = cpool.tile([128, NQ, D], MMDT, name="wo_sb")
        nc.sync.dma_start(wo_sb[:], io["wo"].rearrange("(h p) n -> p h n", p=128))
        cos_sb = cpool.tile([128, T], MMDT, name="cos_sb")
        nc.sync.dma_start(cos_sb[:], io["cosT"][:, :])
        sin_sb = cpool.tile([128, T], MMDT, name="sin_sb")
        nc.sync.dma_start(sin_sb[:], io["sinT"][:, :])
        perm_sb = cpool.tile([128, HD], MMDT, name="perm_sb")
        nc.sync.dma_start(perm_sb[:], io["perm"][:, :])
        qw_sb = cpool.tile([128, 1], F32, name="qw_sb")
        nc.sync.dma_start(qw_sb[:], io["qw_col"][:, :])
        kw_sb = cpool.tile([128, 1], F32, name="kw_sb")
        nc.sync.dma_start(kw_sb[:], io["kw_col"][:, :])
        mask_sb = cpool.tile([128, 4, TB], MMDT, name="mask_sb")
        nc.sync.dma_start(mask_sb[:], io["mask512"].rearrange("o p c -> p o c"))
        onesc_sb = cpool.tile([128, 1], MMDT, name="onesc_sb")
        nc.sync.dma_start(onesc_sb[:], io["ones_col"][:, :])
        eps_sb = cpool.tile([128, 1], F32, name="eps_sb")
        nc.gpsimd.memset(eps_sb[:], EPS)

        # ---------- persistent activations ----------
        qrope = ppool.tile([128, NQ, T], MMDT, name="qrope")
        krope = ppool.tile([128, T], MMDT, name="krope")
        sg = ppool.tile([128, NQ, T], MMDT, name="sg")
        v_sb = ppool.tile([128, NTT, HD], MMDT, name="v_sb")
        attnT = ppool.tile([128, NQ, T], MMDT, name="attnT")

        xT_r = io["xT"].rearrange("(ko p) t -> p ko t", p=128)

        for tb in range(NTB):
            tsl = ds(tb * TB, TB)

            # ======== Phase B: QKV projection + norm + rope ========
            xt = xpool.tile([128, NKT, TB], MMDT, name="xt", tag="xt")
            nc.sync.dma_start(xt[:], xT_r[:, :, tsl])

            # proj specs: (lhsT slice picker, out writer) for q0..q3, k
            qk_specs = []
            for h in range(NQ):
                qk_specs.append(("q", h))
            qk_specs.append(("k", 0))

            # staged tails: per tensor i, PE pieces [var, varrep, rot]
            # emitted 1, 2, 3 accum-blocks after accum(i).
            tails = {}   # i -> dict with intermediate tiles

            def accum_qk(i):
                kind, h = qk_specs[i]
                ps = ps_acc.tile([128, TB], F32, name=f"psqk_{tb}_{i}", tag="acc")
                for kt in range(NKT):
                    if kind == "q":
                        lhsT = wq_sb[:, kt, ts(h, HD)]
                    else:
                        lhsT = wk_sb[:, kt, :]
                    _mm(nc, ps, lhsT, xt[:, kt, :], start=(kt == 0), stop=(kt == NKT - 1))
                # immediate non-PE evac: square + bf16 copy
                sq = wb.tile([128, TB], MMDT, name=f"sq_{tb}_{i}", tag="sq")
                nc.scalar.activation(sq[:], ps[:], AF.Square)
                qsb = wb.tile([128, TB], MMDT, name=f"qsb_{tb}_{i}", tag="qsb")
                w_col = qw_sb if kind == "q" else kw_sb
                nc.vector.tensor_scalar_mul(qsb[:], ps[:], w_col[:, 0:1])
                tails[i] = {"sq": sq, "qsb": qsb, "kind": kind, "h": h}

            def tail_var(i):
                st = tails[i]
                vps = ps_misc.tile([1, TB], F32, name=f"var_{tb}_{i}", tag="misc")
                _mm(nc, vps, onesc_sb[:, :], st["sq"][:, :], start=True, stop=True)
                vrow = rows.tile([1, TB], F32, name=f"vrow_{tb}_{i}", tag="row")
                nc.vector.tensor_copy(vrow[:], vps[:])
                vrep = wb.tile([128, TB], F32, name=f"vrep_{tb}_{i}", tag="vrep")
                nc.gpsimd.partition_broadcast(vrep[:], vrow[0:1, :])
                st["vrep"] = vrep

            def tail_varrep(i):
                st = tails[i]
                rstd = wb.tile([128, TB], F32, name=f"rstd_{tb}_{i}", tag="rstd")
                nc.scalar.activation(rstd[:], st["vrep"][:], AF.Sqrt,
                                     bias=eps_sb[:, 0:1], scale=1.0 / HD)
                nc.vector.reciprocal(rstd[:], rstd[:])
                qn = wb.tile([128, TB], MMDT, name=f"qn_{tb}_{i}", tag="qn")
                nc.vector.tensor_mul(qn[:], st["qsb"][:], rstd[:])
                st["qn"] = qn

            def tail_rot(i):
                st = tails[i]
                rot = ps_misc.tile([128, TB], F32, name=f"rot_{tb}_{i}", tag="misc")
                _mm(nc, rot[:], perm_sb[:, :], st["qn"][:, :], start=True, stop=True)
                t1 = wb.tile([128, TB], F32, name=f"t1_{tb}_{i}", tag="t1")
                nc.vector.tensor_mul(t1[:], st["qn"][:], cos_sb[:, tsl])
                t2 = wb.tile([128, TB], F32, name=f"t2_{tb}_{i}", tag="t2")
                nc.vector.tensor_mul(t2[:], rot[:], sin_sb[:, tsl])
                dst = qrope[:, st["h"], tsl] if st["kind"] == "q" else krope[:, tsl]
                nc.vector.tensor_add(dst, t1[:], t2[:])

            def accum_gate(h):
                ps = ps_acc.tile([128, TB], F32, name=f"psg_{tb}_{h}", tag="acc")
                for kt in range(NKT):
                    _mm(nc, ps, wg_sb[:, kt, ts(h, HD)], xt[:, kt, :],
                        start=(kt == 0), stop=(kt == NKT - 1))
                nc.scalar.activation(sg[:, h, tsl], ps[:], AF.Sigmoid)

            def accum_v(tt):
                ti = tb * 4 + tt
                ps = ps_misc.tile([128, HD], F32, name=f"psv_{tb}_{tt}", tag="misc")
                for kt in range(NKT):
                    _mm(nc, ps, xt[:, kt, ts(tt, 128)], wv_sb[:, kt, :],
                        start=(kt == 0), stop=(kt == NKT - 1))
                nc.vector.tensor_copy(v_sb[:, ti, :], ps[:])

            # PE block sequence with interleaved tails
            blocks = ([lambda i=i: accum_qk(i) for i in range(5)]
                      + [lambda h=h: accum_gate(h) for h in range(NQ)]
                      + [lambda tt=tt: accum_v(tt) for tt in range(4)])
            tail_sched = {}  # block idx -> list of tail fns
            for i in range(5):
                tail_sched.setdefault(i + 1, []).append(lambda i=i: tail_var(i))
                tail_sched.setdefault(i + 2, []).append(lambda i=i: tail_varrep(i))
                tail_sched.setdefault(i + 3, []).append(lambda i=i: tail_rot(i))
            for bi, blk in enumerate(blocks):
                blk()
                for fn in tail_sched.get(bi + 1, ()):
                    fn()

            # ======== Phase C: attention for this query block ========
            nj = 4 * (tb + 1)
            for h in range(NQ):
                attn_ps = ps_acc.tile([128, TB], F32, name=f"attn_{tb}_{h}", tag="acc")
                den_ps = ps_den.tile([1, TB], F32, name=f"den_{tb}_{h}", tag="den")
                probs_t = [None] * nj

                def emit_scores(j, h=h):
                    sp = ps_sc.tile([128, TB], F32, name=f"sc_{tb}_{h}_{j}", tag="sc")
                    _mm(nc, sp, krope[:, ts(j, 128)], qrope[:, h, tsl],
                        start=True, stop=True)
                    pr = prp.tile([128, TB], MMDT, name=f"pr_{tb}_{h}_{j}", tag="pr")
                    nc.scalar.activation(pr[:], sp[:], AF.Exp, scale=SCALE)
                    o = j - 4 * tb
                    if o >= 0:
                        nc.vector.tensor_mul(pr[:], pr[:], mask_sb[:, o, :])
                    probs_t[j] = pr

                def emit_av(j, h=h, nj=nj):
                    pr = probs_t[j]
                    _mm(nc, attn_ps, v_sb[:, j, :], pr[:],
                        start=(j == 0), stop=(j == nj - 1))
                    _mm(nc, den_ps, onesc_sb[:, :], pr[:],
                        start=(j == 0), stop=(j == nj - 1))

                LOOK = 2
                for j in range(nj):
                    emit_scores(j)
                    if j >= LOOK:
                        emit_av(j - LOOK)
                for j in range(max(0, nj - LOOK), nj):
                    emit_av(j)

                # normalize + gate
                drow = rows.tile([1, TB], F32, name=f"drow_{tb}_{h}", tag="row")
                nc.vector.tensor_copy(drow[:], den_ps[:])
                rden = wc.tile([128, TB], F32, name=f"rden_{tb}_{h}", tag="rden")
                nc.gpsimd.partition_broadcast(rden[:], drow[0:1, :])
                nc.vector.reciprocal(rden[:], rden[:])
                g1 = wc.tile([128, TB], F32, name=f"g1_{tb}_{h}", tag="g1")
                nc.vector.tensor_mul(g1[:], attn_ps[:], rden[:])
                nc.vector.tensor_mul(attnT[:, h, tsl], g1[:], sg[:, h, tsl])

            # ======== Phase D: o_proj for this query block ========
            for tt in range(4):
                ti = tb * 4 + tt
                for nb in range(NTB):
                    pso = ps_acc.tile([128, TB], F32, name=f"pso_{ti}_{nb}", tag="acc")
                    for h in range(NQ):
                        _mm(nc, pso, attnT[:, h, ts(ti, 128)],
                            wo_sb[:, h, ts(nb, TB)],
                            start=(h == 0), stop=(h == NQ - 1))
                    osb = op.tile([128, TB], F32, name=f"osb_{ti}_{nb}", tag="osb")
                    nc.scalar.copy(osb[:], pso[:])
                    nc.sync.dma_start(io["out"][ts(ti, 128), ts(nb, TB)], osb[:])


_CACHED = None


def _build():
    global _CACHED
    if _CACHED is not None:
        return _CACHED
    nc = bacc.Bacc("TRN2", target_bir_lowering=False, debug=False, num_devices=8)
    io = {}
    def din(name, shape, dt):
        io[name] = nc.dram_tensor(name, shape, dt, kind="ExternalInput").ap()
    din("xT", [D, T], MMDT)
    din("wq", [D, DH], MMDT)
    din("wg", [D, DH], MMDT)
    din("wk", [D, HD], MMDT)
    din("wv", [D, HD], MMDT)
    din("wo", [DH, D], MMDT)
    din("cosT", [HD, T], MMDT)
    din("sinT", [HD, T], MMDT)
    din("perm", [HD, HD], MMDT)
    din("qw_col", [HD, 1], F32)
    din("kw_col", [HD, 1], F32)
    din("mask512", [4, 128, TB], MMDT)
    din("ones_col", [128, 1], MMDT)
    io["out"] = nc.dram_tensor("out", [T, D], F32, kind="ExternalOutput").ap()

    with tile.TileContext(nc, num_cores=8) as tc:
        _emit(tc, io)
    nc.compile()
    _CACHED = nc
    return nc


def _prep_in_maps(inputs):
    hidden = np.asarray(inputs["hidden_BTD"], np.float32)
    cos = np.asarray(inputs["cos_BTK"], np.float32)
    sin = np.asarray(inputs["sin_BTK"], np.float32)
    w_q = np.asarray(inputs["w_q"], np.float32)
    w_k = np.asarray(inputs["w_k"], np.float32)
    w_v = np.asarray(inputs["w_v"], np.float32)
    w_o = np.asarray(inputs["w_o"], np.float32)
    qw = np.asarray(inputs["q_norm_w"], np.float32)
    kw = np.asarray(inputs["k_norm_w"], np.float32)

    wq4 = w_q.reshape(D, NH, 2 * HD)

    def cvt(x):
        return np.ascontiguousarray(np.asarray(x, np.float32).astype(NPMM))

    # masks: [4][128, 512]; mask[o][jl, c] = 1 iff c >= o*128 + jl
    mask = np.zeros((4, 128, TB), np.float32)
    for o in range(4):
        for jl in range(128):
            mask[o, jl, o * 128 + jl:] = 1.0

    perm = np.zeros((128, 128), np.float32)
    perm[np.arange(64), np.arange(64) + 64] = 1.0
    perm[np.arange(64, 128), np.arange(64, 128) - 64] = -1.0

    in_maps = []
    for c in range(8):
        b, g = divmod(c, 4)
        heads = list(range(4 * g, 4 * g + 4))
        m = {
            "xT": cvt(hidden[b].T),
            "wq": cvt(np.concatenate([wq4[:, h, :HD] for h in heads], axis=1)),
            "wg": cvt(np.concatenate([wq4[:, h, HD:] for h in heads], axis=1)),
            "wk": cvt(w_k[:, g * HD:(g + 1) * HD]),
            "wv": cvt(w_v[:, g * HD:(g + 1) * HD]),
            "wo": cvt(w_o[4 * g * HD:(4 * g + 4) * HD, :]),
            "cosT": cvt(cos[b].T),
            "sinT": cvt(sin[b].T),
            "perm": cvt(perm),
            "qw_col": np.ascontiguousarray(qw[:, None]),
            "kw_col": np.ascontiguousarray(kw[:, None]),
            "mask512": cvt(mask),
            "ones_col": cvt(np.ones((128, 1), np.float32)),
        }
        in_maps.append(m)
    return in_maps


def run(inputs, **spmd_kwargs):
    """Build+run; returns (full_output [B,T,D] fp32, BassKernelResults)."""
    nc = _build()
    in_maps = _prep_in_maps(inputs)
    res = run_bass_kernel_spmd(nc, in_maps, core_ids=list(range(8)), **spmd_kwargs)
    out = np.zeros((B, T, D), np.float32)
    for c in range(8):
        out[c // 4] += res.results[c]["out"]
    return out, res


def kernel(**inputs):
    out, _ = run(inputs)
    return out
